# revision 24
# baseline (speedup 1.0000x reference)
"""Trainium2 Bass kernel for nn_Explore_Recommendation_Decoder.

Computation (B=256, L=50, H=128, N=100000):
  additive attention over L -> ctx -> feat=[ctx,lm] [B,2H]
  logits = feat @ Wexp [B,N]; mask items present in history to -inf
  out = softmax(logits, axis=1)

Sharding (8 cores): pure data-parallel over batch — each core owns 32
batch rows end to end (attention, full-N logits, softmax) and reads the
full Wexp (cast to bf16 on host). ZERO cross-core collectives, so each
core's execution is completely independent of the others' dispatch
times (the previous N-sharded design lost ~78ms to cross-core sync).

SBUF packing: partition 32g+r holds batch row r of N-quarter g, so the
exp results of the [32, 500] PSUM tiles pack all 128 partitions, and
the fused mask-multiply+reduce (tensor_tensor_reduce) and the rescale
run at full 128-lane occupancy. The per-row denominator is a
cross-partition sum over the 4 quarters, done with one matmul against
a constant mod-32 selection matrix. Output is written in the packed
[128, 25000] layout with 20KB-contiguous descriptors; the host unpacks
with one numpy transpose.

Host does only input/output marshaling: transposes/slices, bf16 cast
of Wexp, the item-history 0/1 keep mask as packed int8, and the output
unpack/concat.
"""

import sys
import numpy as np

for _p in ("/opt/trn_rl_repo", "/root/.axon_site/_ro/trn_rl_repo"):
    if _p not in sys.path:
        sys.path.insert(0, _p)

import concourse.bass as bass
import concourse.bacc as bacc
import concourse.mybir as mybir
import concourse.tile as tile
from concourse.bass_utils import run_bass_kernel_spmd

F32 = mybir.dt.float32
BF16 = mybir.dt.bfloat16
I8 = mybir.dt.int8
AF = mybir.ActivationFunctionType
ALU = mybir.AluOpType

B, L, H, N = 256, 50, 128, 100000
NCORES = 8
BC = B // NCORES          # 32 batch rows per core
JC = BC * L               # 1600 = flattened (b, l) for this core's rows
G = 4                     # partition groups = contiguous N-quarters
PC = N // G               # 25000 packed columns per partition (one quarter)
TS = 500                  # psum sub-tile columns (one PSUM bank)
TD = 2500                 # N columns per weight DMA tile
NS = TD // TS             # 5 psum sub-tiles per weight tile
NM = PC // TD             # 10 weight tiles per quarter
SCW = 5000                # rescale/output super-chunk columns

BF16_NP = mybir.dt.np(BF16)

_CACHE = {}


def _build():
    """Build the SPMD Bass program (identical on all 8 cores)."""
    nc = bacc.Bacc(None, target_bir_lowering=False, debug=False,
                   num_devices=NCORES)

    # ---- per-core external inputs -------------------------------------
    amT = nc.dram_tensor("amT", [H, JC], F32, kind="ExternalInput")
    lmT_own = nc.dram_tensor("lmT_own", [H, BC], F32, kind="ExternalInput")
    ue_w = nc.dram_tensor("ue_w", [H, H], F32, kind="ExternalInput")
    we_w = nc.dram_tensor("we_w", [H, H], F32, kind="ExternalInput")
    ve_w = nc.dram_tensor("ve_w", [H, 1], F32, kind="ExternalInput")
    tanh_b = nc.dram_tensor("tanh_b", [H, 1], F32, kind="ExternalInput")
    score_add = nc.dram_tensor("score_add", [1, JC], F32, kind="ExternalInput")
    modsel = nc.dram_tensor("modsel", [H, H], F32, kind="ExternalInput")
    wexp = nc.dram_tensor("wexp", [2 * H, N], BF16, kind="ExternalInput")
    nmask = nc.dram_tensor("nmask", [H, PC], I8, kind="ExternalInput")
    out = nc.dram_tensor("out", [H, PC], F32, kind="ExternalOutput")

    with tile.TileContext(nc) as tc:
        with (
            tc.tile_pool(name="const", bufs=1) as cp,
            tc.tile_pool(name="big", bufs=1) as bp,
            tc.tile_pool(name="wp", bufs=3) as wp,
        ):
            # ---- resident tiles ----------------------------------------
            e_sb = bp.tile([128, PC], F32)           # exp(logits), packed
            nm_sb = bp.tile([128, PC], I8)           # 0/1 keep mask, packed
            sacc = bp.tile([128, NM], F32)           # per-chunk masked-exp sums

            ue_t = cp.tile([H, H], F32)
            nc.sync.dma_start(ue_t[:], ue_w[:, :])
            we_t = cp.tile([H, H], F32)
            nc.sync.dma_start(we_t[:], we_w[:, :])
            ve_t = cp.tile([H, 1], F32)
            nc.sync.dma_start(ve_t[:], ve_w[:, :])
            tb_t = cp.tile([H, 1], F32)
            nc.sync.dma_start(tb_t[:], tanh_b[:, :])
            ms_t = cp.tile([H, H], F32)
            nc.sync.dma_start(ms_t[:], modsel[:, :])
            lmo_t = cp.tile([H, BC], F32)
            nc.sync.dma_start(lmo_t[:], lmT_own[:, :])
            sa_t = cp.tile([1, JC], F32)
            nc.sync.dma_start(sa_t[:], score_add[:, :])
            amT_t = cp.tile([H, JC], F32)
            nc.sync.dma_start(amT_t[:], amT[:, :])
            nc.sync.dma_start(nm_sb[:], nmask[:, :])

            # ---- attention (this core's 32 batch rows) -----------------
            with tc.tile_pool(name="psA", bufs=1, space="PSUM") as pa:
                # qT = We^T @ lmT_own  [k=128, b=32]
                q_ps = pa.tile([H, BC], F32)
                nc.tensor.matmul(q_ps[:], we_t[:], lmo_t[:], start=True, stop=True)
                q_sb = cp.tile([H, BC], F32)
                nc.scalar.copy(q_sb[:], q_ps[:])

                # aT = Ue^T @ amT; pre = aT + qT (broadcast over l)
                pre_sb = cp.tile([H, JC], F32)
                CH = 400                       # 8 batch rows * 50
                for i in range(JC // CH):
                    a_ps = pa.tile([H, CH], F32, tag="a_ps")
                    nc.tensor.matmul(a_ps[:], ue_t[:],
                                     amT_t[:, i * CH:(i + 1) * CH],
                                     start=True, stop=True)
                    qb = q_sb[:, 8 * i:8 * i + 8].unsqueeze(-1) \
                        .broadcast_to([H, 8, L])
                    nc.vector.tensor_tensor(
                        pre_sb[:, i * CH:(i + 1) * CH].rearrange(
                            "p (b l) -> p b l", l=L),
                        a_ps[:].rearrange("p (b l) -> p b l", l=L),
                        qb, ALU.add)

                # t = tanh(pre + (Ue_b + We_b))
                t_sb = cp.tile([H, JC], F32)
                nc.scalar.activation(t_sb[:], pre_sb[:], AF.Tanh,
                                     bias=tb_t[:, 0:1])

                # scores = Ve^T @ t (+ attention mask)  [1, 1600]
                s_sb = cp.tile([1, JC], F32)
                for i in range(JC // CH):
                    sv_ps = pa.tile([1, CH], F32, tag="sv_ps")
                    nc.tensor.matmul(sv_ps[:], ve_t[:],
                                     t_sb[:, i * CH:(i + 1) * CH],
                                     start=True, stop=True)
                    nc.vector.tensor_tensor(
                        s_sb[0:1, i * CH:(i + 1) * CH], sv_ps[:],
                        sa_t[0:1, i * CH:(i + 1) * CH], ALU.add)

                # softmax over l per batch row (rows live on partition 0)
                ea_sb = cp.tile([1, JC], F32)
                nc.scalar.activation(ea_sb[:], s_sb[:], AF.Exp)
                sum_sb = cp.tile([1, BC], F32)
                nc.vector.reduce_sum(
                    sum_sb[:], ea_sb[:].rearrange("p (b l) -> p b l", l=L),
                    axis=mybir.AxisListType.X)
                inv_sb = cp.tile([1, BC], F32)
                nc.vector.reciprocal(inv_sb[:], sum_sb[:])
                at_sb = cp.tile([1, JC], F32)
                nc.vector.tensor_tensor(
                    at_sb[:].rearrange("p (b l) -> p b l", l=L),
                    ea_sb[:].rearrange("p (b l) -> p b l", l=L),
                    inv_sb[:].unsqueeze(-1).broadcast_to([1, BC, L]),
                    ALU.mult)

                # ctxT[h, b] = sum_l amT[h,(b,l)] * attn[(b,l)]
                ones_t = cp.tile([1, H], F32)
                nc.vector.memset(ones_t[:], 1.0)
                prod_sb = cp.tile([H, JC], F32)
                for i in range(JC // CH):
                    bc_ps = pa.tile([H, CH], F32, tag="bc_ps")
                    nc.tensor.matmul(bc_ps[:], ones_t[:],
                                     at_sb[0:1, i * CH:(i + 1) * CH],
                                     start=True, stop=True)
                    nc.vector.tensor_tensor(
                        prod_sb[:, i * CH:(i + 1) * CH],
                        amT_t[:, i * CH:(i + 1) * CH], bc_ps[:], ALU.mult)
                ctxT_sb = cp.tile([H, BC], F32)
                nc.vector.reduce_sum(
                    ctxT_sb[:], prod_sb[:].rearrange("p (b l) -> p b l", l=L),
                    axis=mybir.AxisListType.X)

            # featT halves as bf16 matmul lhsT (no transposes needed)
            f0_bf = cp.tile([H, BC], BF16)
            nc.scalar.copy(f0_bf[:], ctxT_sb[:])
            f1_bf = cp.tile([H, BC], BF16)
            nc.scalar.copy(f1_bf[:], lmo_t[:])

            # ---- big matmul: full-N logits for 32 rows, packed 4x ------
            # group g (partitions 32g..32g+31) owns N-quarter
            # [25000g, 25000(g+1)); chunk-major order so the fused
            # mask+reduce (full 128 lanes) follows each chunk's 4 quarters.
            with tc.tile_pool(name="psB", bufs=6, space="PSUM") as pb:
                for m in range(NM):
                    for g in range(G):
                        n0 = PC * g + TD * m
                        wk0 = wp.tile([128, TD], BF16, tag="wk0")
                        nc.sync.dma_start(wk0[:], wexp[0:128, n0:n0 + TD])
                        wk1 = wp.tile([128, TD], BF16, tag="wk1")
                        nc.sync.dma_start(wk1[:], wexp[128:256, n0:n0 + TD])
                        for s in range(NS):
                            c0 = TS * s
                            pc = TD * m + c0
                            ps = pb.tile([BC, TS], F32, tag="mm")
                            nc.tensor.matmul(ps[:], f0_bf[:],
                                             wk0[:, c0:c0 + TS],
                                             start=True, stop=False)
                            nc.tensor.matmul(ps[:], f1_bf[:],
                                             wk1[:, c0:c0 + TS],
                                             start=False, stop=True)
                            nc.scalar.activation(
                                e_sb[BC * g:BC * (g + 1), pc:pc + TS],
                                ps[:], AF.Exp)
                    # fused 0/1-mask multiply + partial denominator sum,
                    # at full 128-lane occupancy, overlapped with the stream
                    ec = e_sb[:, TD * m:TD * (m + 1)]
                    nc.vector.tensor_tensor(
                        ec, ec, nm_sb[:, TD * m:TD * (m + 1)], ALU.mult)
                    nc.vector.reduce_sum(sacc[:, m:m + 1], ec,
                                         axis=mybir.AxisListType.X)

            # ---- denominators: cross-group (mod-32) sum -----------------
            with tc.tile_pool(name="psD", bufs=1, space="PSUM") as pd:
                sg = bp.tile([128, 1], F32)
                nc.vector.reduce_sum(sg[:], sacc[:], axis=mybir.AxisListType.X)
                ps_s = pd.tile([128, 1], F32)
                nc.tensor.matmul(ps_s[:], ms_t[:], sg[:],
                                 start=True, stop=True)
                inv = bp.tile([128, 1], F32)
                nc.vector.reciprocal(inv[:], ps_s[:])

            # ---- rescale + write out, chunked for overlap ---------------
            # one DMA per chunk, all 128 partitions, written in the packed
            # [128, PC] layout (row 32g+r = batch row r, N-quarter g); the
            # host unpacks to [BC, N] with one cheap numpy transpose
            for k in range(PC // SCW):
                sl = e_sb[:, SCW * k:SCW * (k + 1)]
                nc.vector.tensor_scalar_mul(sl, sl, inv[:, 0:1])
                nc.sync.dma_start(out[0:H, SCW * k:SCW * (k + 1)], sl)

    nc.compile()
    return nc


def _prep_in_maps(all_memory, last_memory, seq_item, mask,
                  Ue_w, Ue_b, We_w, We_b, Ve_w, Ve_b, Wexp):
    am = np.ascontiguousarray(np.asarray(all_memory, np.float32))
    lm = np.asarray(last_memory, np.float32)
    seq = np.asarray(seq_item)
    msk = np.asarray(mask, bool)
    amT_full = np.ascontiguousarray(am.transpose(2, 0, 1))     # [H, B, L]
    lmT = np.ascontiguousarray(lm.T)                           # [H, B]
    score_add_full = np.where(msk, np.float32(-1e9), np.float32(0.0))
    tanh_bias = (np.asarray(Ue_b, np.float32)
                 + np.asarray(We_b, np.float32)).reshape(H, 1)
    ve = np.ascontiguousarray(np.asarray(Ve_w, np.float32).reshape(H, 1))
    ue = np.ascontiguousarray(np.asarray(Ue_w, np.float32))
    we = np.ascontiguousarray(np.asarray(We_w, np.float32))
    wex = np.ascontiguousarray(np.asarray(Wexp, np.float32).astype(BF16_NP))
    kk = np.arange(H)
    modsel_m = ((kk[:, None] % BC) == (kk[None, :] % BC)).astype(np.float32)

    # item-history mask -> 0/1 int8 keep mask (0 on items in the history)
    nm = np.ones((B, N), np.int8)
    valid = seq > 0
    rows = np.broadcast_to(np.arange(B)[:, None], seq.shape)
    nm[rows[valid], seq[valid]] = 0

    in_maps = []
    for c in range(NCORES):
        b0 = BC * c
        # pack [BC, N] -> [128, PC]: partition 32g+r <- row r, N-quarter g
        nm_p = np.ascontiguousarray(
            nm[b0:b0 + BC].reshape(BC, G, PC).transpose(1, 0, 2)
        ).reshape(H, PC)
        in_maps.append({
            "amT": np.ascontiguousarray(
                amT_full[:, b0:b0 + BC, :]).reshape(H, JC),
            "lmT_own": np.ascontiguousarray(lmT[:, b0:b0 + BC]),
            "ue_w": ue,
            "we_w": we,
            "ve_w": ve,
            "tanh_b": tanh_bias,
            "score_add": np.ascontiguousarray(
                score_add_full[b0:b0 + BC, :]).reshape(1, JC),
            "modsel": modsel_m,
            "wexp": wex,
            "nmask": nm_p,
        })
    return in_maps


def _get_nc():
    if "nc" not in _CACHE:
        _CACHE["nc"] = _build()
    return _CACHE["nc"]


def run(in_maps, **kwargs):
    return run_bass_kernel_spmd(_get_nc(), in_maps, list(range(NCORES)),
                                **kwargs)


def _unpack_out(arr):
    """[128, PC] packed (row 32g+r = batch row r, N-quarter g) -> [BC, N]."""
    return np.ascontiguousarray(
        arr.reshape(G, BC, PC).transpose(1, 0, 2)).reshape(BC, N)


def kernel(**inputs):
    in_maps = _prep_in_maps(**inputs)
    res = run(in_maps)
    return np.concatenate(
        [_unpack_out(res.results[c]["out"]) for c in range(NCORES)], axis=0)


# revision 26
# speedup vs baseline: 1.0684x; 1.0684x over previous
"""Trainium2 Bass kernel for nn_Explore_Recommendation_Decoder.

Computation (B=256, L=50, H=128, N=100000):
  additive attention over L -> ctx -> feat=[ctx,lm] [B,2H]
  logits = feat @ Wexp [B,N]; mask items present in history to -inf
  out = softmax(logits, axis=1)

Sharding (8 cores): pure data-parallel over batch — each core owns 32
batch rows end to end (attention, full-N logits, softmax) and reads the
full Wexp (cast to bf16 on host). ZERO cross-core collectives, so each
core's execution is completely independent of the others' dispatch
times (the previous N-sharded design lost ~78ms to cross-core sync).

SBUF packing: partition 32g+r holds batch row r of N-quarter g, so the
exp results of the [32, 500] PSUM tiles pack all 128 partitions, and
the fused mask-multiply+reduce (tensor_tensor_reduce) and the rescale
run at full 128-lane occupancy. The per-row denominator is a
cross-partition sum over the 4 quarters, done with one matmul against
a constant mod-32 selection matrix. Output is written in the packed
[128, 25000] layout with 20KB-contiguous descriptors; the host unpacks
with one numpy transpose.

Host does only input/output marshaling: transposes/slices, bf16 cast
of Wexp, the item-history 0/1 keep mask as packed int8, and the output
unpack/concat.
"""

import sys
import numpy as np

for _p in ("/opt/trn_rl_repo", "/root/.axon_site/_ro/trn_rl_repo"):
    if _p not in sys.path:
        sys.path.insert(0, _p)

import concourse.bass as bass
import concourse.bacc as bacc
import concourse.mybir as mybir
import concourse.tile as tile
from concourse.bass_utils import run_bass_kernel_spmd

F32 = mybir.dt.float32
BF16 = mybir.dt.bfloat16
I8 = mybir.dt.int8
AF = mybir.ActivationFunctionType
ALU = mybir.AluOpType

B, L, H, N = 256, 50, 128, 100000
NCORES = 8
BC = B // NCORES          # 32 batch rows per core
JC = BC * L               # 1600 = flattened (b, l) for this core's rows
G = 4                     # partition groups = contiguous N-quarters
PC = N // G               # 25000 packed columns per partition (one quarter)
TS = 500                  # psum sub-tile columns (one PSUM bank)
TD = 2500                 # N columns per weight DMA tile
NS = TD // TS             # 5 psum sub-tiles per weight tile
NM = PC // TD             # 10 weight tiles per quarter
SCW = 5000                # rescale/output super-chunk columns

BF16_NP = mybir.dt.np(BF16)

_CACHE = {}


def _build():
    """Build the SPMD Bass program (identical on all 8 cores)."""
    nc = bacc.Bacc(None, target_bir_lowering=False, debug=False,
                   num_devices=NCORES)

    # ---- per-core external inputs -------------------------------------
    amT = nc.dram_tensor("amT", [H, JC], F32, kind="ExternalInput")
    lmT_own = nc.dram_tensor("lmT_own", [H, BC], F32, kind="ExternalInput")
    ue_w = nc.dram_tensor("ue_w", [H, H], F32, kind="ExternalInput")
    we_w = nc.dram_tensor("we_w", [H, H], F32, kind="ExternalInput")
    ve_w = nc.dram_tensor("ve_w", [H, 1], F32, kind="ExternalInput")
    tanh_b = nc.dram_tensor("tanh_b", [H, 1], F32, kind="ExternalInput")
    score_add = nc.dram_tensor("score_add", [1, JC], F32, kind="ExternalInput")
    modsel = nc.dram_tensor("modsel", [H, H], F32, kind="ExternalInput")
    wexp = nc.dram_tensor("wexp", [2 * H, N], BF16, kind="ExternalInput")
    nmask = nc.dram_tensor("nmask", [H, PC], I8, kind="ExternalInput")
    out = nc.dram_tensor("out", [H, PC], F32, kind="ExternalOutput")

    with tile.TileContext(nc) as tc:
        with (
            tc.tile_pool(name="const", bufs=1) as cp,
            tc.tile_pool(name="big", bufs=1) as bp,
            tc.tile_pool(name="wp", bufs=3) as wp,
        ):
            # ---- resident tiles ----------------------------------------
            e_sb = bp.tile([128, PC], F32)           # exp(logits), packed
            nm_sb = bp.tile([128, PC], I8)           # 0/1 keep mask, packed
            sacc = bp.tile([128, NM], F32)           # per-chunk masked-exp sums

            ue_t = cp.tile([H, H], F32)
            nc.gpsimd.dma_start(ue_t[:], ue_w[:, :])
            we_t = cp.tile([H, H], F32)
            nc.gpsimd.dma_start(we_t[:], we_w[:, :])
            ve_t = cp.tile([H, 1], F32)
            nc.gpsimd.dma_start(ve_t[:], ve_w[:, :])
            tb_t = cp.tile([H, 1], F32)
            nc.gpsimd.dma_start(tb_t[:], tanh_b[:, :])
            ms_t = cp.tile([H, H], F32)
            nc.gpsimd.dma_start(ms_t[:], modsel[:, :])
            lmo_t = cp.tile([H, BC], F32)
            nc.gpsimd.dma_start(lmo_t[:], lmT_own[:, :])
            sa_t = cp.tile([1, JC], F32)
            nc.gpsimd.dma_start(sa_t[:], score_add[:, :])
            amT_t = cp.tile([H, JC], F32)
            nc.gpsimd.dma_start(amT_t[:], amT[:, :])
            nc.gpsimd.dma_start(nm_sb[:], nmask[:, :])

            # ---- attention (this core's 32 batch rows) -----------------
            with tc.tile_pool(name="psA", bufs=1, space="PSUM") as pa:
                # qT = We^T @ lmT_own  [k=128, b=32]
                q_ps = pa.tile([H, BC], F32)
                nc.tensor.matmul(q_ps[:], we_t[:], lmo_t[:], start=True, stop=True)
                q_sb = cp.tile([H, BC], F32)
                nc.scalar.copy(q_sb[:], q_ps[:])

                # aT = Ue^T @ amT; pre = aT + qT (broadcast over l)
                pre_sb = cp.tile([H, JC], F32)
                CH = 400                       # 8 batch rows * 50
                for i in range(JC // CH):
                    a_ps = pa.tile([H, CH], F32, tag="a_ps")
                    nc.tensor.matmul(a_ps[:], ue_t[:],
                                     amT_t[:, i * CH:(i + 1) * CH],
                                     start=True, stop=True)
                    qb = q_sb[:, 8 * i:8 * i + 8].unsqueeze(-1) \
                        .broadcast_to([H, 8, L])
                    nc.vector.tensor_tensor(
                        pre_sb[:, i * CH:(i + 1) * CH].rearrange(
                            "p (b l) -> p b l", l=L),
                        a_ps[:].rearrange("p (b l) -> p b l", l=L),
                        qb, ALU.add)

                # t = tanh(pre + (Ue_b + We_b))
                t_sb = cp.tile([H, JC], F32)
                nc.scalar.activation(t_sb[:], pre_sb[:], AF.Tanh,
                                     bias=tb_t[:, 0:1])

                # scores = Ve^T @ t (+ attention mask)  [1, 1600]
                s_sb = cp.tile([1, JC], F32)
                for i in range(JC // CH):
                    sv_ps = pa.tile([1, CH], F32, tag="sv_ps")
                    nc.tensor.matmul(sv_ps[:], ve_t[:],
                                     t_sb[:, i * CH:(i + 1) * CH],
                                     start=True, stop=True)
                    nc.vector.tensor_tensor(
                        s_sb[0:1, i * CH:(i + 1) * CH], sv_ps[:],
                        sa_t[0:1, i * CH:(i + 1) * CH], ALU.add)

                # softmax over l per batch row (rows live on partition 0)
                ea_sb = cp.tile([1, JC], F32)
                nc.scalar.activation(ea_sb[:], s_sb[:], AF.Exp)
                sum_sb = cp.tile([1, BC], F32)
                nc.vector.reduce_sum(
                    sum_sb[:], ea_sb[:].rearrange("p (b l) -> p b l", l=L),
                    axis=mybir.AxisListType.X)
                inv_sb = cp.tile([1, BC], F32)
                nc.vector.reciprocal(inv_sb[:], sum_sb[:])
                at_sb = cp.tile([1, JC], F32)
                nc.vector.tensor_tensor(
                    at_sb[:].rearrange("p (b l) -> p b l", l=L),
                    ea_sb[:].rearrange("p (b l) -> p b l", l=L),
                    inv_sb[:].unsqueeze(-1).broadcast_to([1, BC, L]),
                    ALU.mult)

                # ctxT[h, b] = sum_l amT[h,(b,l)] * attn[(b,l)]
                ones_t = cp.tile([1, H], F32)
                nc.vector.memset(ones_t[:], 1.0)
                prod_sb = cp.tile([H, JC], F32)
                for i in range(JC // CH):
                    bc_ps = pa.tile([H, CH], F32, tag="bc_ps")
                    nc.tensor.matmul(bc_ps[:], ones_t[:],
                                     at_sb[0:1, i * CH:(i + 1) * CH],
                                     start=True, stop=True)
                    nc.vector.tensor_tensor(
                        prod_sb[:, i * CH:(i + 1) * CH],
                        amT_t[:, i * CH:(i + 1) * CH], bc_ps[:], ALU.mult)
                ctxT_sb = cp.tile([H, BC], F32)
                nc.vector.reduce_sum(
                    ctxT_sb[:], prod_sb[:].rearrange("p (b l) -> p b l", l=L),
                    axis=mybir.AxisListType.X)

            # featT halves as bf16 matmul lhsT (no transposes needed)
            f0_bf = cp.tile([H, BC], BF16)
            nc.scalar.copy(f0_bf[:], ctxT_sb[:])
            f1_bf = cp.tile([H, BC], BF16)
            nc.scalar.copy(f1_bf[:], lmo_t[:])

            # ---- big matmul: full-N logits for 32 rows, packed 4x ------
            # group g (partitions 32g..32g+31) owns N-quarter
            # [25000g, 25000(g+1)); chunk-major order so the fused
            # mask+reduce (full 128 lanes) follows each chunk's 4 quarters.
            with tc.tile_pool(name="psB", bufs=6, space="PSUM") as pb:
                for m in range(NM):
                    for g in range(G):
                        n0 = PC * g + TD * m
                        wk0 = wp.tile([128, TD], BF16, tag="wk0")
                        nc.sync.dma_start(wk0[:], wexp[0:128, n0:n0 + TD])
                        wk1 = wp.tile([128, TD], BF16, tag="wk1")
                        nc.sync.dma_start(wk1[:], wexp[128:256, n0:n0 + TD])
                        for s in range(NS):
                            c0 = TS * s
                            pc = TD * m + c0
                            ps = pb.tile([BC, TS], F32, tag="mm")
                            nc.tensor.matmul(ps[:], f0_bf[:],
                                             wk0[:, c0:c0 + TS],
                                             start=True, stop=False)
                            nc.tensor.matmul(ps[:], f1_bf[:],
                                             wk1[:, c0:c0 + TS],
                                             start=False, stop=True)
                            nc.scalar.activation(
                                e_sb[BC * g:BC * (g + 1), pc:pc + TS],
                                ps[:], AF.Exp)
                    # fused 0/1-mask multiply + partial denominator sum,
                    # at full 128-lane occupancy, overlapped with the stream
                    ec = e_sb[:, TD * m:TD * (m + 1)]
                    nc.vector.tensor_tensor(
                        ec, ec, nm_sb[:, TD * m:TD * (m + 1)], ALU.mult)
                    nc.vector.reduce_sum(sacc[:, m:m + 1], ec,
                                         axis=mybir.AxisListType.X)

            # ---- denominators: cross-group (mod-32) sum -----------------
            with tc.tile_pool(name="psD", bufs=1, space="PSUM") as pd:
                sg = bp.tile([128, 1], F32)
                nc.vector.reduce_sum(sg[:], sacc[:], axis=mybir.AxisListType.X)
                ps_s = pd.tile([128, 1], F32)
                nc.tensor.matmul(ps_s[:], ms_t[:], sg[:],
                                 start=True, stop=True)
                inv = bp.tile([128, 1], F32)
                nc.vector.reciprocal(inv[:], ps_s[:])

            # ---- rescale + write out, chunked for overlap ---------------
            # one DMA per chunk, all 128 partitions, written in the packed
            # [128, PC] layout (row 32g+r = batch row r, N-quarter g); the
            # host unpacks to [BC, N] with one cheap numpy transpose
            for k in range(PC // SCW):
                sl = e_sb[:, SCW * k:SCW * (k + 1)]
                eng = nc.vector if k % 2 == 0 else nc.gpsimd
                eng.tensor_scalar_mul(sl, sl, inv[:, 0:1])
                dma_eng = nc.sync if k % 2 == 0 else nc.scalar
                dma_eng.dma_start(out[0:H, SCW * k:SCW * (k + 1)], sl)

    nc.compile()
    return nc


def _prep_in_maps(all_memory, last_memory, seq_item, mask,
                  Ue_w, Ue_b, We_w, We_b, Ve_w, Ve_b, Wexp):
    am = np.ascontiguousarray(np.asarray(all_memory, np.float32))
    lm = np.asarray(last_memory, np.float32)
    seq = np.asarray(seq_item)
    msk = np.asarray(mask, bool)
    amT_full = np.ascontiguousarray(am.transpose(2, 0, 1))     # [H, B, L]
    lmT = np.ascontiguousarray(lm.T)                           # [H, B]
    score_add_full = np.where(msk, np.float32(-1e9), np.float32(0.0))
    tanh_bias = (np.asarray(Ue_b, np.float32)
                 + np.asarray(We_b, np.float32)).reshape(H, 1)
    ve = np.ascontiguousarray(np.asarray(Ve_w, np.float32).reshape(H, 1))
    ue = np.ascontiguousarray(np.asarray(Ue_w, np.float32))
    we = np.ascontiguousarray(np.asarray(We_w, np.float32))
    wex = np.ascontiguousarray(np.asarray(Wexp, np.float32).astype(BF16_NP))
    kk = np.arange(H)
    modsel_m = ((kk[:, None] % BC) == (kk[None, :] % BC)).astype(np.float32)

    # item-history mask -> 0/1 int8 keep mask (0 on items in the history)
    nm = np.ones((B, N), np.int8)
    valid = seq > 0
    rows = np.broadcast_to(np.arange(B)[:, None], seq.shape)
    nm[rows[valid], seq[valid]] = 0

    in_maps = []
    for c in range(NCORES):
        b0 = BC * c
        # pack [BC, N] -> [128, PC]: partition 32g+r <- row r, N-quarter g
        nm_p = np.ascontiguousarray(
            nm[b0:b0 + BC].reshape(BC, G, PC).transpose(1, 0, 2)
        ).reshape(H, PC)
        in_maps.append({
            "amT": np.ascontiguousarray(
                amT_full[:, b0:b0 + BC, :]).reshape(H, JC),
            "lmT_own": np.ascontiguousarray(lmT[:, b0:b0 + BC]),
            "ue_w": ue,
            "we_w": we,
            "ve_w": ve,
            "tanh_b": tanh_bias,
            "score_add": np.ascontiguousarray(
                score_add_full[b0:b0 + BC, :]).reshape(1, JC),
            "modsel": modsel_m,
            "wexp": wex,
            "nmask": nm_p,
        })
    return in_maps


def _get_nc():
    if "nc" not in _CACHE:
        _CACHE["nc"] = _build()
    return _CACHE["nc"]


def run(in_maps, **kwargs):
    return run_bass_kernel_spmd(_get_nc(), in_maps, list(range(NCORES)),
                                **kwargs)


def _unpack_out(arr):
    """[128, PC] packed (row 32g+r = batch row r, N-quarter g) -> [BC, N]."""
    return np.ascontiguousarray(
        arr.reshape(G, BC, PC).transpose(1, 0, 2)).reshape(BC, N)


def kernel(**inputs):
    in_maps = _prep_in_maps(**inputs)
    res = run(in_maps)
    return np.concatenate(
        [_unpack_out(res.results[c]["out"]) for c in range(NCORES)], axis=0)
